# revision 18
# baseline (speedup 1.0000x reference)
r"""GCN block (gather -> normalize -> scatter-add -> linear -> relu) on 8 trn2 cores.

Math: out = relu( \hat{A} (X W) + b ) with \hat{A} = D^-1/2 (A + I) D^-1/2,
degree over destination of (edges + self loops).

v3 "materialized identity-stream" design:
  The norm factorizes: norm(e) = dinv[src] * dinv[dst]. Fold dinv[src] into a
  host-prescaled table x' = dinv[:,None] * x (fp16), and dinv[dst] into a
  per-window constant diagonal rhs. Self loops become ordinary messages
  (src == dst, rank 0 of each dst).

  Host routing (per core, 12500 dst nodes = 98 windows of 128):
   - message m = k-th in-message of dst d (self loop first). If k < T (=14),
     m rides IDENTITY chunk k of d's window at slot = d%128: the scatter
     matmul rhs is the CONSTANT diag(dinv of the window's dsts), so no
     per-chunk DVE build and no per-message index on the device.
   - k >= T messages go to per-window OVERFLOW chunks (dense, any slot) with
     a classic one-hot rhs (iota==dst_off)*dinv[dst] built by tensor_scalar.
   - The whole message stream (identity + overflow chunk slots, zero rows for
     padding) is MATERIALIZED on the host, transposed to stream_t
     [128 slots, C*128 ch] fp16, so the device "gather" is a plain sequential
     HWDGE dma_start per PSUM group (~2.4MB each, full HBM bandwidth; no
     SWDGE descriptor-issue bottleneck, which limited the previous design to
     ~1.25ms at ~1.42us per 128-descriptor indirect-DMA call).

  Device per PSUM group (4 windows = one 512-col PSUM bank):
   - 1 dma_start pulls the group's chunk slab into SBUF
   - per window: diag rhs built once (tensor_scalar, Pool), T identity
     matmuls + K_w overflow matmuls accumulate ps1[ch, dst] (PE, fp16,
     128 cycles each)
   - epilogue: ps1 -> fp16 agg (DVE copy), ps2 = W^T-form matmul, relu+bias
     on ACT, DMA out [ch, dst]; host transposes back.

Program shape depends only on the cross-core per-window overflow chunk
counts (k-table); identity chunk count T is fixed.
"""

import sys
from contextlib import ExitStack
from dataclasses import dataclass

import numpy as np

if "/opt/trn_rl_repo" not in sys.path:
    sys.path.insert(0, "/opt/trn_rl_repo")

import concourse.bass as bass
import concourse.bacc as bacc
import concourse.mybir as mybir
import concourse.tile as tile
from concourse.bass_utils import run_bass_kernel_spmd


def _ensure_axon_hooks_stub():
    """The image's antenv package lacks axon_hooks; bass_utils imports it on
    the trace path (e.g. when BASS_TRACE is set). Provide a stub returning
    None so tracing degrades gracefully instead of raising ImportError."""
    import types

    name = "antenv.axon_hooks"
    if name in sys.modules:
        return
    try:
        __import__(name)
        return
    except ImportError:
        pass
    mod = types.ModuleType(name)
    mod._hook = None
    mod.set_axon_ntff_profile_hook = lambda h: setattr(mod, "_hook", h)
    mod.get_axon_ntff_profile_hook = lambda: mod._hook
    sys.modules[name] = mod
    try:
        import antenv

        antenv.axon_hooks = mod
    except ImportError:
        pass


_ensure_axon_hooks_stub()

P = 128
T_ID = 14  # identity chunks per window (covers the first T_ID msgs of each dst)
GRP = 4  # windows per PSUM group


@dataclass(frozen=True)
class Cfg:
    n_nodes: int = 100000
    in_ch: int = 128
    out_ch: int = 128
    m: int = 8  # cores

    @property
    def np_per(self) -> int:
        return self.n_nodes // self.m

    @property
    def n_win(self) -> int:
        return (self.np_per + P - 1) // P


FULL = Cfg()


def route_edges(edge_index: np.ndarray, cfg: Cfg = FULL):
    """Host-side routing (indices only). Returns (k_ovf, per_core):
    k_ovf[w] = overflow chunks for window w (max over cores, len n_win);
    per_core[p] = dict of index arrays for make_in_maps:
      id_col/id_slot/id_src  — stream position of each identity message
      ov_col/ov_slot/ov_src/ov_off/ov_dinv — same for overflow messages
      (cols are *local* chunk ids before k-table padding: filled in later)
      plus dinv (full-table) for the caller."""
    n = cfg.n_nodes
    nw = cfg.n_win
    src = np.asarray(edge_index[0], dtype=np.int64)
    dst = np.asarray(edge_index[1], dtype=np.int64)

    deg = (np.bincount(dst, minlength=n) + 1).astype(np.float32)
    dinv = (1.0 / np.sqrt(deg, dtype=np.float32)).astype(np.float32)

    # messages = self loops first (rank 0 within each dst), then edges
    loop = np.arange(n, dtype=np.int64)
    msrc = np.concatenate([loop, src])
    mdst = np.concatenate([loop, dst])
    order = np.argsort(mdst, kind="stable")
    s_dst = mdst[order]
    s_src = msrc[order]
    # rank of each message within its dst (loops got rank 0)
    starts = np.searchsorted(s_dst, np.arange(n))
    rank = np.arange(len(s_dst), dtype=np.int64) - starts[s_dst]

    per_core = []
    k_real = np.zeros((cfg.m, nw), np.int64)
    for p in range(cfg.m):
        base = p * cfg.np_per
        lo = np.searchsorted(s_dst, base)
        hi = np.searchsorted(s_dst, base + cfg.np_per)
        d_loc = s_dst[lo:hi] - base
        c_src = s_src[lo:hi]
        c_rank = rank[lo:hi]
        w = d_loc >> 7
        slot = d_loc & 127

        idm = c_rank < T_ID
        id_w = w[idm]
        id_chunk = c_rank[idm]  # chunk-in-window (0..T_ID-1)
        id_slot = slot[idm]
        id_src = c_src[idm]

        ovm = ~idm
        ov_w = w[ovm]  # sorted ascending (messages sorted by dst)
        ov_src = c_src[ovm]
        ov_dst = d_loc[ovm]
        wstart = np.searchsorted(ov_w, np.arange(nw))
        pos = np.arange(len(ov_w), dtype=np.int64) - wstart[ov_w]
        ov_chunk = pos >> 7
        ov_slot = pos & 127
        k_real[p] = np.ceil(np.bincount(ov_w, minlength=nw) / P).astype(np.int64)

        per_core.append(
            dict(
                id_w=id_w,
                id_chunk=id_chunk,
                id_slot=id_slot,
                id_src=id_src,
                ov_w=ov_w,
                ov_chunk=ov_chunk,
                ov_slot=ov_slot,
                ov_src=ov_src,
                ov_off=(ov_dst & 127),
                ov_dinv=dinv[ov_dst + base],
            )
        )

    k_ovf = k_real.max(axis=0)  # [n_win]
    # s_dst/s_src kept for the post-run sample check in kernel()
    per_core.append(dict(s_dst=s_dst, s_src=s_src))
    return k_ovf, per_core, dinv


def build_program(k_ovf, cfg: Cfg = FULL, sdt=mybir.dt.float16):
    """Build + compile the SPMD bass program (identical on all cores)."""
    nw = cfg.n_win
    k_ovf = np.asarray(k_ovf, dtype=np.int64)
    c_tot = int(nw * T_ID + k_ovf.sum())
    c_ovf = int(k_ovf.sum())
    n_grp = (nw + GRP - 1) // GRP

    nc = bacc.Bacc(
        "TRN2",
        target_bir_lowering=False,
        debug=False,
        enable_asserts=False,
        num_devices=cfg.m,
    )
    f32 = mybir.dt.float32
    stream_t = nc.dram_tensor("stream_t", [P, c_tot * P], sdt, kind="ExternalInput").ap()
    do_in = nc.dram_tensor("do_ovf", [P, max(c_ovf, 1)], f32, kind="ExternalInput").ap()
    nv_in = nc.dram_tensor("nv_ovf", [P, max(c_ovf, 1)], f32, kind="ExternalInput").ap()
    d2_in = nc.dram_tensor("d2", [P, nw], f32, kind="ExternalInput").ap()
    io_in = nc.dram_tensor("iota", [P, P], sdt, kind="ExternalInput").ap()
    ioc_in = nc.dram_tensor("iotac", [P, 1], f32, kind="ExternalInput").ap()
    w_in = nc.dram_tensor("w", [cfg.in_ch, cfg.out_ch], sdt, kind="ExternalInput").ap()
    b_in = nc.dram_tensor("b", [P, 1], f32, kind="ExternalInput").ap()
    out_t = nc.dram_tensor("out_t", [P, nw * P], sdt, kind="ExternalOutput").ap()

    with tile.TileContext(nc) as tc:
        with ExitStack() as ctx:
            cpool = ctx.enter_context(tc.tile_pool(name="const", bufs=1))
            gpool = ctx.enter_context(tc.tile_pool(name="gather", bufs=6))
            ohpool = ctx.enter_context(tc.tile_pool(name="oh", bufs=24))
            aggpool = ctx.enter_context(tc.tile_pool(name="agg", bufs=4))
            outpool = ctx.enter_context(tc.tile_pool(name="outp", bufs=4))
            pp1 = ctx.enter_context(tc.tile_pool(name="ps1", bufs=4, space="PSUM"))
            pp2 = ctx.enter_context(tc.tile_pool(name="ps2", bufs=2, space="PSUM"))

            do = cpool.tile([P, max(c_ovf, 1)], f32)
            nv = cpool.tile([P, max(c_ovf, 1)], f32)
            d2 = cpool.tile([P, nw], f32)
            io = cpool.tile([P, P], sdt)
            ioc = cpool.tile([P, 1], f32)
            wt = cpool.tile([P, cfg.out_ch], sdt)
            bb = cpool.tile([P, 1], f32)
            nc.sync.dma_start(out=do[:], in_=do_in[:])
            nc.sync.dma_start(out=nv[:], in_=nv_in[:])
            nc.sync.dma_start(out=d2[:], in_=d2_in[:])
            nc.sync.dma_start(out=io[:], in_=io_in[:])
            nc.sync.dma_start(out=ioc[:], in_=ioc_in[:])
            nc.sync.dma_start(out=wt[:], in_=w_in[:])
            nc.sync.dma_start(out=bb[:], in_=b_in[:])

            # pre-build every window's diag rhs up front: the bulk of PE work
            # (identity matmuls) then depends only on the stream slab DMA
            diag_all = cpool.tile([P, nw * P], sdt)
            for w in range(nw):
                nc.vector.tensor_scalar(
                    out=diag_all[:, w * P : (w + 1) * P],
                    in0=io[:],
                    scalar1=ioc[:],
                    scalar2=d2[:, w : w + 1],
                    op0=mybir.AluOpType.is_equal,
                    op1=mybir.AluOpType.mult,
                )

            col = 0  # stream chunk column
            colk = 0  # overflow table column
            for gi in range(n_grp):
                wls = list(range(gi * GRP, min((gi + 1) * GRP, nw)))
                gw = len(wls) * P
                kg = sum(T_ID + int(k_ovf[w]) for w in wls)
                gt = gpool.tile([P, kg * P], sdt)
                # alternate the two HWDGE engines (SP / ACT) so back-to-back
                # stream slabs overlap their fixed DGE overheads
                deng = nc.sync if gi % 2 == 0 else nc.scalar
                deng.dma_start(
                    out=gt[:], in_=stream_t[:, col * P : (col + kg) * P]
                )
                ps1 = pp1.tile([P, gw], mybir.dt.float32, space="PSUM")
                cc = 0
                for wl, w in enumerate(wls):
                    kw = int(k_ovf[w])
                    for k in range(T_ID):
                        nc.tensor.matmul(
                            ps1[:, wl * P : (wl + 1) * P],
                            lhsT=gt[:, cc * P : (cc + 1) * P],
                            rhs=diag_all[:, w * P : (w + 1) * P],
                            start=(k == 0),
                            stop=(k == T_ID - 1 and kw == 0),
                        )
                        cc += 1
                    for c in range(kw):
                        oh = ohpool.tile([P, P], sdt)
                        nc.vector.tensor_scalar(
                            out=oh[:],
                            in0=io[:],
                            scalar1=do[:, colk + c : colk + c + 1],
                            scalar2=nv[:, colk + c : colk + c + 1],
                            op0=mybir.AluOpType.is_equal,
                            op1=mybir.AluOpType.mult,
                        )
                        nc.tensor.matmul(
                            ps1[:, wl * P : (wl + 1) * P],
                            lhsT=gt[:, cc * P : (cc + 1) * P],
                            rhs=oh[:],
                            start=False,
                            stop=(c == kw - 1),
                        )
                        cc += 1
                    colk += kw
                col += kg
                agg = aggpool.tile([P, gw], sdt)
                nc.scalar.copy(out=agg[:], in_=ps1[:])
                ps2 = pp2.tile([P, gw], mybir.dt.float32, space="PSUM")
                nc.tensor.matmul(ps2[:], lhsT=wt[:], rhs=agg[:], start=True, stop=True)
                ot = outpool.tile([P, gw], sdt)
                nc.scalar.activation(
                    out=ot[:],
                    in_=ps2[:],
                    func=mybir.ActivationFunctionType.Relu,
                    bias=bb[:],
                    scale=1.0,
                )
                (nc.scalar if gi % 2 == 0 else nc.sync).dma_start(
                    out=out_t[:, wls[0] * P : (wls[0] + len(wls)) * P], in_=ot[:]
                )

    nc.compile()
    return nc


def make_in_maps(x, W, b, k_ovf, per_core, dinv, cfg: Cfg = FULL, np_sdt=np.float16):
    nw = cfg.n_win
    k_ovf = np.asarray(k_ovf, dtype=np.int64)
    c_tot = int(nw * T_ID + k_ovf.sum())
    c_ovf = int(k_ovf.sum())
    # column base of window w's identity block in the stream; overflow block
    # follows immediately. Also the overflow-table column base per window.
    cumk = np.zeros(nw + 1, np.int64)
    np.cumsum(k_ovf, out=cumk[1:])
    col_base = T_ID * np.arange(nw, dtype=np.int64) + cumk[:-1]
    ovf_base = cumk[:-1]

    x32 = np.asarray(x, dtype=np.float32)
    x2 = (x32 * dinv[:, None]).astype(np_sdt)  # dinv[src]-prescaled table

    iota = np.broadcast_to(
        np.arange(P, dtype=np.float32), (P, P)
    ).astype(np_sdt).copy()
    iotac = np.arange(P, dtype=np.float32).reshape(P, 1).copy()
    w_np = np.ascontiguousarray(np.asarray(W, dtype=np.float32)).astype(np_sdt)
    b_np = np.asarray(b, dtype=np.float32).reshape(P, 1).copy()

    in_maps = []
    for p in range(cfg.m):
        r = per_core[p]
        base = p * cfg.np_per
        stream = np.zeros((c_tot, P, cfg.in_ch), np_sdt)
        icol = col_base[r["id_w"]] + r["id_chunk"]
        stream[icol, r["id_slot"]] = x2[r["id_src"]]
        ocol = col_base[r["ov_w"]] + T_ID + r["ov_chunk"]
        stream[ocol, r["ov_slot"]] = x2[r["ov_src"]]
        stream_t = np.ascontiguousarray(
            stream.transpose(1, 0, 2).reshape(P, c_tot * cfg.in_ch)
        )

        do_np = np.zeros((P, max(c_ovf, 1)), np.float32)
        nv_np = np.zeros((P, max(c_ovf, 1)), np.float32)
        okol = ovf_base[r["ov_w"]] + r["ov_chunk"]
        do_np[r["ov_slot"], okol] = r["ov_off"].astype(np.float32)
        nv_np[r["ov_slot"], okol] = r["ov_dinv"]

        d2_np = np.zeros((P, nw), np.float32)
        nn = cfg.np_per
        loc = np.arange(nn, dtype=np.int64)
        d2_np[loc & 127, loc >> 7] = dinv[base + loc]

        in_maps.append(
            dict(
                stream_t=stream_t,
                do_ovf=do_np,
                nv_ovf=nv_np,
                d2=d2_np,
                iota=iota,
                iotac=iotac,
                w=w_np,
                b=b_np,
            )
        )
    return in_maps


_PROG_CACHE = {}


def _sample_check(out, x, W, b, dinv, s_dst, s_src, n_samples=512, seed=7):
    """Host-recompute a random sample of output rows; returns True if the
    device output matches (guards against rare first-run DMA/engine races)."""
    n = out.shape[0]
    rng = np.random.default_rng(seed)
    samp = rng.choice(n, size=n_samples, replace=False)
    x32 = np.asarray(x, dtype=np.float32)
    w32 = np.asarray(W, dtype=np.float32)
    b32 = np.asarray(b, dtype=np.float32)
    starts = np.searchsorted(s_dst, samp)
    ends = np.searchsorted(s_dst, samp + 1)
    for d, lo, hi in zip(samp, starts, ends):
        srcs = s_src[lo:hi]
        agg = (x32[srcs] * dinv[srcs][:, None]).sum(axis=0) * dinv[d]
        exp = np.maximum(agg @ w32 + b32, 0.0)
        scale = max(float(np.linalg.norm(exp)), 1e-3)
        if float(np.linalg.norm(out[d] - exp)) > 0.02 * scale:
            return False
    return True


def kernel(x, edge_index, W, b):
    cfg = FULL
    k_ovf, per_core, dinv = route_edges(edge_index, cfg)
    aux = per_core[cfg.m]  # s_dst/s_src appended by route_edges
    key = (tuple(int(v) for v in k_ovf), cfg)
    if key not in _PROG_CACHE:
        _PROG_CACHE[key] = build_program(k_ovf, cfg)
    nc = _PROG_CACHE[key]
    in_maps = make_in_maps(x, W, b, k_ovf, per_core, dinv, cfg)
    out = np.empty((cfg.n_nodes, cfg.out_ch), np.float32)
    for attempt in range(3):
        res = run_bass_kernel_spmd(nc, in_maps, core_ids=list(range(cfg.m)))
        for p in range(cfg.m):
            out[p * cfg.np_per : (p + 1) * cfg.np_per] = (
                res.results[p]["out_t"][:, : cfg.np_per].T.astype(np.float32)
            )
        if _sample_check(out, x, W, b, dinv, aux["s_dst"], aux["s_src"]):
            break
        print(f"kernel: sample check failed (attempt {attempt}), re-running", flush=True)
    return out


# revision 20
# speedup vs baseline: 1.1388x; 1.1388x over previous
r"""GCN block (gather -> normalize -> scatter-add -> linear -> relu) on 8 trn2 cores.

Math: out = relu( \hat{A} (X W) + b ) with \hat{A} = D^-1/2 (A + I) D^-1/2,
degree over destination of (edges + self loops).

v3 "materialized identity-stream" design:
  The norm factorizes: norm(e) = dinv[src] * dinv[dst]. Fold dinv[src] into a
  host-prescaled table x' = dinv[:,None] * x (fp16), and dinv[dst] into a
  per-window constant diagonal rhs. Self loops become ordinary messages
  (src == dst, rank 0 of each dst).

  Host routing (per core, 12500 dst nodes = 98 windows of 128):
   - message m = k-th in-message of dst d (self loop first). If k < T (=14),
     m rides IDENTITY chunk k of d's window at slot = d%128: the scatter
     matmul rhs is the CONSTANT diag(dinv of the window's dsts), so no
     per-chunk DVE build and no per-message index on the device.
   - k >= T messages go to per-window OVERFLOW chunks (dense, any slot) with
     a classic one-hot rhs (iota==dst_off)*dinv[dst] built by tensor_scalar.
   - The whole message stream (identity + overflow chunk slots, zero rows for
     padding) is MATERIALIZED on the host, transposed to stream_t
     [128 slots, C*128 ch] fp16, so the device "gather" is a plain sequential
     HWDGE dma_start per PSUM group (~2.4MB each, full HBM bandwidth; no
     SWDGE descriptor-issue bottleneck, which limited the previous design to
     ~1.25ms at ~1.42us per 128-descriptor indirect-DMA call).

  Device per PSUM group (4 windows = one 512-col PSUM bank):
   - 1 dma_start pulls the group's chunk slab into SBUF
   - per window: diag rhs built once (tensor_scalar, Pool), T identity
     matmuls + K_w overflow matmuls accumulate ps1[ch, dst] (PE, fp16,
     128 cycles each)
   - epilogue: ps1 -> fp16 agg (DVE copy), ps2 = W^T-form matmul, relu+bias
     on ACT, DMA out [ch, dst]; host transposes back.

Program shape depends only on the cross-core per-window overflow chunk
counts (k-table); identity chunk count T is fixed.
"""

import sys
from contextlib import ExitStack
from dataclasses import dataclass

import numpy as np

if "/opt/trn_rl_repo" not in sys.path:
    sys.path.insert(0, "/opt/trn_rl_repo")

import concourse.bass as bass
import concourse.bacc as bacc
import concourse.mybir as mybir
import concourse.tile as tile
from concourse.bass_utils import run_bass_kernel_spmd


def _ensure_axon_hooks_stub():
    """The image's antenv package lacks axon_hooks; bass_utils imports it on
    the trace path (e.g. when BASS_TRACE is set). Provide a stub returning
    None so tracing degrades gracefully instead of raising ImportError."""
    import types

    name = "antenv.axon_hooks"
    if name in sys.modules:
        return
    try:
        __import__(name)
        return
    except ImportError:
        pass
    mod = types.ModuleType(name)
    mod._hook = None
    mod.set_axon_ntff_profile_hook = lambda h: setattr(mod, "_hook", h)
    mod.get_axon_ntff_profile_hook = lambda: mod._hook
    sys.modules[name] = mod
    try:
        import antenv

        antenv.axon_hooks = mod
    except ImportError:
        pass


_ensure_axon_hooks_stub()

P = 128
T_ID = 14  # identity chunks per window (covers the first T_ID msgs of each dst)
GRP = 4  # windows per PSUM group


@dataclass(frozen=True)
class Cfg:
    n_nodes: int = 100000
    in_ch: int = 128
    out_ch: int = 128
    m: int = 8  # cores

    @property
    def np_per(self) -> int:
        return self.n_nodes // self.m

    @property
    def n_win(self) -> int:
        return (self.np_per + P - 1) // P


FULL = Cfg()


def route_edges(edge_index: np.ndarray, cfg: Cfg = FULL):
    """Host-side routing (indices only). Returns (k_ovf, per_core):
    k_ovf[w] = overflow chunks for window w (max over cores, len n_win);
    per_core[p] = dict of index arrays for make_in_maps:
      id_col/id_slot/id_src  — stream position of each identity message
      ov_col/ov_slot/ov_src/ov_off/ov_dinv — same for overflow messages
      (cols are *local* chunk ids before k-table padding: filled in later)
      plus dinv (full-table) for the caller."""
    n = cfg.n_nodes
    nw = cfg.n_win
    src = np.asarray(edge_index[0], dtype=np.int64)
    dst = np.asarray(edge_index[1], dtype=np.int64)

    deg = (np.bincount(dst, minlength=n) + 1).astype(np.float32)
    dinv = (1.0 / np.sqrt(deg, dtype=np.float32)).astype(np.float32)

    # messages = self loops first (rank 0 within each dst), then edges
    loop = np.arange(n, dtype=np.int64)
    msrc = np.concatenate([loop, src])
    mdst = np.concatenate([loop, dst])
    order = np.argsort(mdst, kind="stable")
    s_dst = mdst[order]
    s_src = msrc[order]
    # rank of each message within its dst (loops got rank 0)
    starts = np.searchsorted(s_dst, np.arange(n))
    rank = np.arange(len(s_dst), dtype=np.int64) - starts[s_dst]

    per_core = []
    k_real = np.zeros((cfg.m, nw), np.int64)
    for p in range(cfg.m):
        base = p * cfg.np_per
        lo = np.searchsorted(s_dst, base)
        hi = np.searchsorted(s_dst, base + cfg.np_per)
        d_loc = s_dst[lo:hi] - base
        c_src = s_src[lo:hi]
        c_rank = rank[lo:hi]
        w = d_loc >> 7
        slot = d_loc & 127

        idm = c_rank < T_ID
        id_w = w[idm]
        id_chunk = c_rank[idm]  # chunk-in-window (0..T_ID-1)
        id_slot = slot[idm]
        id_src = c_src[idm]

        ovm = ~idm
        ov_w = w[ovm]  # sorted ascending (messages sorted by dst)
        ov_src = c_src[ovm]
        ov_dst = d_loc[ovm]
        wstart = np.searchsorted(ov_w, np.arange(nw))
        pos = np.arange(len(ov_w), dtype=np.int64) - wstart[ov_w]
        ov_chunk = pos >> 7
        ov_slot = pos & 127
        k_real[p] = np.ceil(np.bincount(ov_w, minlength=nw) / P).astype(np.int64)

        per_core.append(
            dict(
                id_w=id_w,
                id_chunk=id_chunk,
                id_slot=id_slot,
                id_src=id_src,
                ov_w=ov_w,
                ov_chunk=ov_chunk,
                ov_slot=ov_slot,
                ov_src=ov_src,
                ov_off=(ov_dst & 127),
                ov_dinv=dinv[ov_dst + base],
            )
        )

    k_ovf = k_real.max(axis=0)  # [n_win]
    # s_dst/s_src kept for the post-run sample check in kernel()
    per_core.append(dict(s_dst=s_dst, s_src=s_src))
    return k_ovf, per_core, dinv


def build_program(k_ovf, cfg: Cfg = FULL, sdt=mybir.dt.float16):
    """Build + compile the SPMD bass program (identical on all cores)."""
    nw = cfg.n_win
    k_ovf = np.asarray(k_ovf, dtype=np.int64)
    c_tot = int(nw * T_ID + k_ovf.sum())
    c_ovf = int(k_ovf.sum())
    n_grp = (nw + GRP - 1) // GRP

    nc = bacc.Bacc(
        "TRN2",
        target_bir_lowering=False,
        debug=False,
        enable_asserts=False,
        num_devices=cfg.m,
    )
    f32 = mybir.dt.float32
    stream_t = nc.dram_tensor("stream_t", [P, c_tot * P], sdt, kind="ExternalInput").ap()
    do_in = nc.dram_tensor("do_ovf", [P, max(c_ovf, 1)], f32, kind="ExternalInput").ap()
    nv_in = nc.dram_tensor("nv_ovf", [P, max(c_ovf, 1)], f32, kind="ExternalInput").ap()
    d2_in = nc.dram_tensor("d2", [P, nw], f32, kind="ExternalInput").ap()
    io_in = nc.dram_tensor("iota", [P, P], sdt, kind="ExternalInput").ap()
    ioc_in = nc.dram_tensor("iotac", [P, 1], f32, kind="ExternalInput").ap()
    w_in = nc.dram_tensor("w", [cfg.in_ch, cfg.out_ch], sdt, kind="ExternalInput").ap()
    b_in = nc.dram_tensor("b", [P, 1], f32, kind="ExternalInput").ap()
    out_t = nc.dram_tensor("out_t", [P, nw * P], sdt, kind="ExternalOutput").ap()

    with tile.TileContext(nc) as tc:
        with ExitStack() as ctx:
            cpool = ctx.enter_context(tc.tile_pool(name="const", bufs=1))
            gpool = ctx.enter_context(tc.tile_pool(name="gather", bufs=6))
            ohpool = ctx.enter_context(tc.tile_pool(name="oh", bufs=24))
            aggpool = ctx.enter_context(tc.tile_pool(name="agg", bufs=4))
            outpool = ctx.enter_context(tc.tile_pool(name="outp", bufs=4))
            pp1 = ctx.enter_context(tc.tile_pool(name="ps1", bufs=4, space="PSUM"))
            pp2 = ctx.enter_context(tc.tile_pool(name="ps2", bufs=2, space="PSUM"))

            do = cpool.tile([P, max(c_ovf, 1)], f32)
            nv = cpool.tile([P, max(c_ovf, 1)], f32)
            d2 = cpool.tile([P, nw], f32)
            io = cpool.tile([P, P], sdt)
            ioc = cpool.tile([P, 1], f32)
            wt = cpool.tile([P, cfg.out_ch], sdt)
            bb = cpool.tile([P, 1], f32)
            nc.sync.dma_start(out=do[:], in_=do_in[:])
            nc.sync.dma_start(out=nv[:], in_=nv_in[:])
            nc.sync.dma_start(out=d2[:], in_=d2_in[:])
            nc.sync.dma_start(out=io[:], in_=io_in[:])
            nc.sync.dma_start(out=ioc[:], in_=ioc_in[:])
            nc.sync.dma_start(out=wt[:], in_=w_in[:])
            nc.sync.dma_start(out=bb[:], in_=b_in[:])

            # pre-build every window's diag rhs up front as separate tiles
            # (one dep each): the bulk of PE work (identity matmuls) then
            # depends only on the stream slab DMA
            dgpool = ctx.enter_context(tc.tile_pool(name="dg", bufs=nw))
            diag_w = []
            for w in range(nw):
                dgt = dgpool.tile([P, P], sdt)
                nc.vector.tensor_scalar(
                    out=dgt[:],
                    in0=io[:],
                    scalar1=ioc[:],
                    scalar2=d2[:, w : w + 1],
                    op0=mybir.AluOpType.is_equal,
                    op1=mybir.AluOpType.mult,
                )
                diag_w.append(dgt)

            col = 0  # stream chunk column
            colk = 0  # overflow table column
            for gi in range(n_grp):
                wls = list(range(gi * GRP, min((gi + 1) * GRP, nw)))
                gw = len(wls) * P
                kg = sum(T_ID + int(k_ovf[w]) for w in wls)
                gt = gpool.tile([P, kg * P], sdt)
                # alternate the two HWDGE engines (SP / ACT) so back-to-back
                # stream slabs overlap their fixed DGE overheads
                deng = nc.sync if gi % 2 == 0 else nc.scalar
                deng.dma_start(
                    out=gt[:], in_=stream_t[:, col * P : (col + kg) * P]
                )
                ps1 = pp1.tile([P, gw], mybir.dt.float32, space="PSUM")
                cc = 0
                for wl, w in enumerate(wls):
                    kw = int(k_ovf[w])
                    for k in range(T_ID):
                        nc.tensor.matmul(
                            ps1[:, wl * P : (wl + 1) * P],
                            lhsT=gt[:, cc * P : (cc + 1) * P],
                            rhs=diag_w[w][:],
                            start=(k == 0),
                            stop=(k == T_ID - 1 and kw == 0),
                        )
                        cc += 1
                    for c in range(kw):
                        oh = ohpool.tile([P, P], sdt)
                        nc.vector.tensor_scalar(
                            out=oh[:],
                            in0=io[:],
                            scalar1=do[:, colk + c : colk + c + 1],
                            scalar2=nv[:, colk + c : colk + c + 1],
                            op0=mybir.AluOpType.is_equal,
                            op1=mybir.AluOpType.mult,
                        )
                        nc.tensor.matmul(
                            ps1[:, wl * P : (wl + 1) * P],
                            lhsT=gt[:, cc * P : (cc + 1) * P],
                            rhs=oh[:],
                            start=False,
                            stop=(c == kw - 1),
                        )
                        cc += 1
                    colk += kw
                col += kg
                agg = aggpool.tile([P, gw], sdt)
                nc.scalar.copy(out=agg[:], in_=ps1[:])
                ps2 = pp2.tile([P, gw], mybir.dt.float32, space="PSUM")
                nc.tensor.matmul(ps2[:], lhsT=wt[:], rhs=agg[:], start=True, stop=True)
                ot = outpool.tile([P, gw], sdt)
                nc.scalar.activation(
                    out=ot[:],
                    in_=ps2[:],
                    func=mybir.ActivationFunctionType.Relu,
                    bias=bb[:],
                    scale=1.0,
                )
                (nc.scalar if gi % 2 == 0 else nc.sync).dma_start(
                    out=out_t[:, wls[0] * P : (wls[0] + len(wls)) * P], in_=ot[:]
                )

    nc.compile()
    return nc


def make_in_maps(x, W, b, k_ovf, per_core, dinv, cfg: Cfg = FULL, np_sdt=np.float16):
    nw = cfg.n_win
    k_ovf = np.asarray(k_ovf, dtype=np.int64)
    c_tot = int(nw * T_ID + k_ovf.sum())
    c_ovf = int(k_ovf.sum())
    # column base of window w's identity block in the stream; overflow block
    # follows immediately. Also the overflow-table column base per window.
    cumk = np.zeros(nw + 1, np.int64)
    np.cumsum(k_ovf, out=cumk[1:])
    col_base = T_ID * np.arange(nw, dtype=np.int64) + cumk[:-1]
    ovf_base = cumk[:-1]

    x32 = np.asarray(x, dtype=np.float32)
    x2 = (x32 * dinv[:, None]).astype(np_sdt)  # dinv[src]-prescaled table

    iota = np.broadcast_to(
        np.arange(P, dtype=np.float32), (P, P)
    ).astype(np_sdt).copy()
    iotac = np.arange(P, dtype=np.float32).reshape(P, 1).copy()
    w_np = np.ascontiguousarray(np.asarray(W, dtype=np.float32)).astype(np_sdt)
    b_np = np.asarray(b, dtype=np.float32).reshape(P, 1).copy()

    in_maps = []
    for p in range(cfg.m):
        r = per_core[p]
        base = p * cfg.np_per
        stream = np.zeros((c_tot, P, cfg.in_ch), np_sdt)
        icol = col_base[r["id_w"]] + r["id_chunk"]
        stream[icol, r["id_slot"]] = x2[r["id_src"]]
        ocol = col_base[r["ov_w"]] + T_ID + r["ov_chunk"]
        stream[ocol, r["ov_slot"]] = x2[r["ov_src"]]
        stream_t = np.ascontiguousarray(
            stream.transpose(1, 0, 2).reshape(P, c_tot * cfg.in_ch)
        )

        do_np = np.zeros((P, max(c_ovf, 1)), np.float32)
        nv_np = np.zeros((P, max(c_ovf, 1)), np.float32)
        okol = ovf_base[r["ov_w"]] + r["ov_chunk"]
        do_np[r["ov_slot"], okol] = r["ov_off"].astype(np.float32)
        nv_np[r["ov_slot"], okol] = r["ov_dinv"]

        d2_np = np.zeros((P, nw), np.float32)
        nn = cfg.np_per
        loc = np.arange(nn, dtype=np.int64)
        d2_np[loc & 127, loc >> 7] = dinv[base + loc]

        in_maps.append(
            dict(
                stream_t=stream_t,
                do_ovf=do_np,
                nv_ovf=nv_np,
                d2=d2_np,
                iota=iota,
                iotac=iotac,
                w=w_np,
                b=b_np,
            )
        )
    return in_maps


_PROG_CACHE = {}


def _sample_check(out, x, W, b, dinv, s_dst, s_src, n_samples=512, seed=7):
    """Host-recompute a random sample of output rows; returns True if the
    device output matches (guards against rare first-run DMA/engine races)."""
    n = out.shape[0]
    rng = np.random.default_rng(seed)
    samp = rng.choice(n, size=n_samples, replace=False)
    x32 = np.asarray(x, dtype=np.float32)
    w32 = np.asarray(W, dtype=np.float32)
    b32 = np.asarray(b, dtype=np.float32)
    starts = np.searchsorted(s_dst, samp)
    ends = np.searchsorted(s_dst, samp + 1)
    for d, lo, hi in zip(samp, starts, ends):
        srcs = s_src[lo:hi]
        agg = (x32[srcs] * dinv[srcs][:, None]).sum(axis=0) * dinv[d]
        exp = np.maximum(agg @ w32 + b32, 0.0)
        scale = max(float(np.linalg.norm(exp)), 1e-3)
        if float(np.linalg.norm(out[d] - exp)) > 0.02 * scale:
            return False
    return True


def kernel(x, edge_index, W, b):
    cfg = FULL
    k_ovf, per_core, dinv = route_edges(edge_index, cfg)
    aux = per_core[cfg.m]  # s_dst/s_src appended by route_edges
    key = (tuple(int(v) for v in k_ovf), cfg)
    if key not in _PROG_CACHE:
        _PROG_CACHE[key] = build_program(k_ovf, cfg)
    nc = _PROG_CACHE[key]
    in_maps = make_in_maps(x, W, b, k_ovf, per_core, dinv, cfg)
    out = np.empty((cfg.n_nodes, cfg.out_ch), np.float32)
    for attempt in range(3):
        res = run_bass_kernel_spmd(nc, in_maps, core_ids=list(range(cfg.m)))
        for p in range(cfg.m):
            out[p * cfg.np_per : (p + 1) * cfg.np_per] = (
                res.results[p]["out_t"][:, : cfg.np_per].T.astype(np.float32)
            )
        if _sample_check(out, x, W, b, dinv, aux["s_dst"], aux["s_src"]):
            break
        print(f"kernel: sample check failed (attempt {attempt}), re-running", flush=True)
    return out


# revision 22
# speedup vs baseline: 1.1517x; 1.0113x over previous
r"""GCN block (gather -> normalize -> scatter-add -> linear -> relu) on 8 trn2 cores.

Math: out = relu( \hat{A} (X W) + b ) with \hat{A} = D^-1/2 (A + I) D^-1/2,
degree over destination of (edges + self loops).

v3 "materialized identity-stream" design:
  The norm factorizes: norm(e) = dinv[src] * dinv[dst]. Fold dinv[src] into a
  host-prescaled table x' = dinv[:,None] * x (fp16), and dinv[dst] into a
  per-window constant diagonal rhs. Self loops become ordinary messages
  (src == dst, rank 0 of each dst).

  Host routing (per core, 12500 dst nodes = 98 windows of 128):
   - message m = k-th in-message of dst d (self loop first). If k < T (=14),
     m rides IDENTITY chunk k of d's window at slot = d%128: the scatter
     matmul rhs is the CONSTANT diag(dinv of the window's dsts), so no
     per-chunk DVE build and no per-message index on the device.
   - k >= T messages go to per-window OVERFLOW chunks (dense, any slot) with
     a classic one-hot rhs (iota==dst_off)*dinv[dst] built by tensor_scalar.
   - The whole message stream (identity + overflow chunk slots, zero rows for
     padding) is MATERIALIZED on the host, transposed to stream_t
     [128 slots, C*128 ch] fp16, so the device "gather" is a plain sequential
     HWDGE dma_start per PSUM group (~2.4MB each, full HBM bandwidth; no
     SWDGE descriptor-issue bottleneck, which limited the previous design to
     ~1.25ms at ~1.42us per 128-descriptor indirect-DMA call).

  Device per PSUM group (4 windows = one 512-col PSUM bank):
   - 1 dma_start pulls the group's chunk slab into SBUF
   - per window: diag rhs built once (tensor_scalar, Pool), T identity
     matmuls + K_w overflow matmuls accumulate ps1[ch, dst] (PE, fp16,
     128 cycles each)
   - epilogue: ps1 -> fp16 agg (DVE copy), ps2 = W^T-form matmul, relu+bias
     on ACT, DMA out [ch, dst]; host transposes back.

Program shape depends only on the cross-core per-window overflow chunk
counts (k-table); identity chunk count T is fixed.
"""

import sys
from contextlib import ExitStack
from dataclasses import dataclass

import numpy as np

if "/opt/trn_rl_repo" not in sys.path:
    sys.path.insert(0, "/opt/trn_rl_repo")

import concourse.bass as bass
import concourse.bacc as bacc
import concourse.mybir as mybir
import concourse.tile as tile
from concourse.bass_utils import run_bass_kernel_spmd


def _ensure_axon_hooks_stub():
    """The image's antenv package lacks axon_hooks; bass_utils imports it on
    the trace path (e.g. when BASS_TRACE is set). Provide a stub returning
    None so tracing degrades gracefully instead of raising ImportError."""
    import types

    name = "antenv.axon_hooks"
    if name in sys.modules:
        return
    try:
        __import__(name)
        return
    except ImportError:
        pass
    mod = types.ModuleType(name)
    mod._hook = None
    mod.set_axon_ntff_profile_hook = lambda h: setattr(mod, "_hook", h)
    mod.get_axon_ntff_profile_hook = lambda: mod._hook
    sys.modules[name] = mod
    try:
        import antenv

        antenv.axon_hooks = mod
    except ImportError:
        pass


_ensure_axon_hooks_stub()

P = 128
T_ID = 14  # identity chunks per window (covers the first T_ID msgs of each dst)
GRP = 4  # windows per PSUM group


@dataclass(frozen=True)
class Cfg:
    n_nodes: int = 100000
    in_ch: int = 128
    out_ch: int = 128
    m: int = 8  # cores

    @property
    def np_per(self) -> int:
        return self.n_nodes // self.m

    @property
    def n_win(self) -> int:
        return (self.np_per + P - 1) // P


FULL = Cfg()


def route_edges(edge_index: np.ndarray, cfg: Cfg = FULL):
    """Host-side routing (indices only). Returns (k_ovf, per_core):
    k_ovf[w] = overflow chunks for window w (max over cores, len n_win);
    per_core[p] = dict of index arrays for make_in_maps:
      id_col/id_slot/id_src  — stream position of each identity message
      ov_col/ov_slot/ov_src/ov_off/ov_dinv — same for overflow messages
      (cols are *local* chunk ids before k-table padding: filled in later)
      plus dinv (full-table) for the caller."""
    n = cfg.n_nodes
    nw = cfg.n_win
    src = np.asarray(edge_index[0], dtype=np.int64)
    dst = np.asarray(edge_index[1], dtype=np.int64)

    deg = (np.bincount(dst, minlength=n) + 1).astype(np.float32)
    dinv = (1.0 / np.sqrt(deg, dtype=np.float32)).astype(np.float32)

    # messages = self loops first (rank 0 within each dst), then edges
    loop = np.arange(n, dtype=np.int64)
    msrc = np.concatenate([loop, src])
    mdst = np.concatenate([loop, dst])
    order = np.argsort(mdst, kind="stable")
    s_dst = mdst[order]
    s_src = msrc[order]
    # rank of each message within its dst (loops got rank 0)
    starts = np.searchsorted(s_dst, np.arange(n))
    rank = np.arange(len(s_dst), dtype=np.int64) - starts[s_dst]

    per_core = []
    k_real = np.zeros((cfg.m, nw), np.int64)
    for p in range(cfg.m):
        base = p * cfg.np_per
        lo = np.searchsorted(s_dst, base)
        hi = np.searchsorted(s_dst, base + cfg.np_per)
        d_loc = s_dst[lo:hi] - base
        c_src = s_src[lo:hi]
        c_rank = rank[lo:hi]
        w = d_loc >> 7
        slot = d_loc & 127

        idm = c_rank < T_ID
        id_w = w[idm]
        id_chunk = c_rank[idm]  # chunk-in-window (0..T_ID-1)
        id_slot = slot[idm]
        id_src = c_src[idm]

        ovm = ~idm
        ov_w = w[ovm]  # sorted ascending (messages sorted by dst)
        ov_src = c_src[ovm]
        ov_dst = d_loc[ovm]
        wstart = np.searchsorted(ov_w, np.arange(nw))
        pos = np.arange(len(ov_w), dtype=np.int64) - wstart[ov_w]
        ov_chunk = pos >> 7
        ov_slot = pos & 127
        k_real[p] = np.ceil(np.bincount(ov_w, minlength=nw) / P).astype(np.int64)

        per_core.append(
            dict(
                id_w=id_w,
                id_chunk=id_chunk,
                id_slot=id_slot,
                id_src=id_src,
                ov_w=ov_w,
                ov_chunk=ov_chunk,
                ov_slot=ov_slot,
                ov_src=ov_src,
                ov_off=(ov_dst & 127),
                ov_dinv=dinv[ov_dst + base],
            )
        )

    k_ovf = k_real.max(axis=0)  # [n_win]
    # s_dst/s_src kept for the post-run sample check in kernel()
    per_core.append(dict(s_dst=s_dst, s_src=s_src))
    return k_ovf, per_core, dinv


def build_program(k_ovf, cfg: Cfg = FULL, sdt=mybir.dt.float16):
    """Build + compile the SPMD bass program (identical on all cores)."""
    nw = cfg.n_win
    k_ovf = np.asarray(k_ovf, dtype=np.int64)
    c_tot = int(nw * T_ID + k_ovf.sum())
    c_ovf = int(k_ovf.sum())
    n_grp = (nw + GRP - 1) // GRP

    nc = bacc.Bacc(
        "TRN2",
        target_bir_lowering=False,
        debug=False,
        enable_asserts=False,
        num_devices=cfg.m,
    )
    f32 = mybir.dt.float32
    stream_t = nc.dram_tensor("stream_t", [P, c_tot * P], sdt, kind="ExternalInput").ap()
    do_in = nc.dram_tensor("do_ovf", [P, max(c_ovf, 1)], f32, kind="ExternalInput").ap()
    nv_in = nc.dram_tensor("nv_ovf", [P, max(c_ovf, 1)], f32, kind="ExternalInput").ap()
    d2_in = nc.dram_tensor("d2", [P, nw], f32, kind="ExternalInput").ap()
    io_in = nc.dram_tensor("iota", [P, P], sdt, kind="ExternalInput").ap()
    ioc_in = nc.dram_tensor("iotac", [P, 1], f32, kind="ExternalInput").ap()
    w_in = nc.dram_tensor("w", [cfg.in_ch, cfg.out_ch], sdt, kind="ExternalInput").ap()
    b_in = nc.dram_tensor("b", [P, 1], f32, kind="ExternalInput").ap()
    out_t = nc.dram_tensor("out_t", [P, nw * P], sdt, kind="ExternalOutput").ap()

    with tile.TileContext(nc) as tc:
        with ExitStack() as ctx:
            cpool = ctx.enter_context(tc.tile_pool(name="const", bufs=1))
            gpool = ctx.enter_context(tc.tile_pool(name="gather", bufs=6))
            ohpool = ctx.enter_context(tc.tile_pool(name="oh", bufs=24))
            aggpool = ctx.enter_context(tc.tile_pool(name="agg", bufs=4))
            outpool = ctx.enter_context(tc.tile_pool(name="outp", bufs=4))
            pp1 = ctx.enter_context(tc.tile_pool(name="ps1", bufs=4, space="PSUM"))
            pp2 = ctx.enter_context(tc.tile_pool(name="ps2", bufs=2, space="PSUM"))

            do = cpool.tile([P, max(c_ovf, 1)], f32)
            nv = cpool.tile([P, max(c_ovf, 1)], f32)
            d2 = cpool.tile([P, nw], f32)
            io = cpool.tile([P, P], sdt)
            ioc = cpool.tile([P, 1], f32)
            wt = cpool.tile([P, cfg.out_ch], sdt)
            bb = cpool.tile([P, 1], f32)
            nc.sync.dma_start(out=do[:], in_=do_in[:])
            nc.sync.dma_start(out=nv[:], in_=nv_in[:])
            nc.sync.dma_start(out=d2[:], in_=d2_in[:])
            nc.sync.dma_start(out=io[:], in_=io_in[:])
            nc.sync.dma_start(out=ioc[:], in_=ioc_in[:])
            nc.sync.dma_start(out=wt[:], in_=w_in[:])
            nc.sync.dma_start(out=bb[:], in_=b_in[:])

            # one diag tile per window, built just-in-time (inline, so DVE
            # program order interleaves diag and overflow-oh builds); bufs=nw
            # so tiles are never recycled (no WAR serialization)
            dgpool = ctx.enter_context(tc.tile_pool(name="dg", bufs=nw))

            col = 0  # stream chunk column
            colk = 0  # overflow table column
            for gi in range(n_grp):
                wls = list(range(gi * GRP, min((gi + 1) * GRP, nw)))
                gw = len(wls) * P
                kg = sum(T_ID + int(k_ovf[w]) for w in wls)
                gt = gpool.tile([P, kg * P], sdt)
                # alternate the two HWDGE engines (SP / ACT) so back-to-back
                # stream slabs overlap their fixed DGE overheads
                deng = nc.sync if gi % 2 == 0 else nc.scalar
                deng.dma_start(
                    out=gt[:], in_=stream_t[:, col * P : (col + kg) * P]
                )
                ps1 = pp1.tile([P, gw], mybir.dt.float32, space="PSUM")
                cc = 0
                for wl, w in enumerate(wls):
                    kw = int(k_ovf[w])
                    dgt = dgpool.tile([P, P], sdt)
                    nc.vector.tensor_scalar(
                        out=dgt[:],
                        in0=io[:],
                        scalar1=ioc[:],
                        scalar2=d2[:, w : w + 1],
                        op0=mybir.AluOpType.is_equal,
                        op1=mybir.AluOpType.mult,
                    )
                    for k in range(T_ID):
                        nc.tensor.matmul(
                            ps1[:, wl * P : (wl + 1) * P],
                            lhsT=gt[:, cc * P : (cc + 1) * P],
                            rhs=dgt[:],
                            start=(k == 0),
                            stop=(k == T_ID - 1 and kw == 0),
                        )
                        cc += 1
                    for c in range(kw):
                        oh = ohpool.tile([P, P], sdt)
                        nc.vector.tensor_scalar(
                            out=oh[:],
                            in0=io[:],
                            scalar1=do[:, colk + c : colk + c + 1],
                            scalar2=nv[:, colk + c : colk + c + 1],
                            op0=mybir.AluOpType.is_equal,
                            op1=mybir.AluOpType.mult,
                        )
                        nc.tensor.matmul(
                            ps1[:, wl * P : (wl + 1) * P],
                            lhsT=gt[:, cc * P : (cc + 1) * P],
                            rhs=oh[:],
                            start=False,
                            stop=(c == kw - 1),
                        )
                        cc += 1
                    colk += kw
                col += kg
                agg = aggpool.tile([P, gw], sdt)
                nc.scalar.copy(out=agg[:], in_=ps1[:])
                ps2 = pp2.tile([P, gw], mybir.dt.float32, space="PSUM")
                nc.tensor.matmul(ps2[:], lhsT=wt[:], rhs=agg[:], start=True, stop=True)
                ot = outpool.tile([P, gw], sdt)
                nc.scalar.activation(
                    out=ot[:],
                    in_=ps2[:],
                    func=mybir.ActivationFunctionType.Relu,
                    bias=bb[:],
                    scale=1.0,
                )
                (nc.scalar if gi % 2 == 0 else nc.sync).dma_start(
                    out=out_t[:, wls[0] * P : (wls[0] + len(wls)) * P], in_=ot[:]
                )

    nc.compile()
    return nc


def make_in_maps(x, W, b, k_ovf, per_core, dinv, cfg: Cfg = FULL, np_sdt=np.float16):
    nw = cfg.n_win
    k_ovf = np.asarray(k_ovf, dtype=np.int64)
    c_tot = int(nw * T_ID + k_ovf.sum())
    c_ovf = int(k_ovf.sum())
    # column base of window w's identity block in the stream; overflow block
    # follows immediately. Also the overflow-table column base per window.
    cumk = np.zeros(nw + 1, np.int64)
    np.cumsum(k_ovf, out=cumk[1:])
    col_base = T_ID * np.arange(nw, dtype=np.int64) + cumk[:-1]
    ovf_base = cumk[:-1]

    x32 = np.asarray(x, dtype=np.float32)
    x2 = (x32 * dinv[:, None]).astype(np_sdt)  # dinv[src]-prescaled table

    iota = np.broadcast_to(
        np.arange(P, dtype=np.float32), (P, P)
    ).astype(np_sdt).copy()
    iotac = np.arange(P, dtype=np.float32).reshape(P, 1).copy()
    w_np = np.ascontiguousarray(np.asarray(W, dtype=np.float32)).astype(np_sdt)
    b_np = np.asarray(b, dtype=np.float32).reshape(P, 1).copy()

    in_maps = []
    for p in range(cfg.m):
        r = per_core[p]
        base = p * cfg.np_per
        stream = np.zeros((c_tot, P, cfg.in_ch), np_sdt)
        icol = col_base[r["id_w"]] + r["id_chunk"]
        stream[icol, r["id_slot"]] = x2[r["id_src"]]
        ocol = col_base[r["ov_w"]] + T_ID + r["ov_chunk"]
        stream[ocol, r["ov_slot"]] = x2[r["ov_src"]]
        stream_t = np.ascontiguousarray(
            stream.transpose(1, 0, 2).reshape(P, c_tot * cfg.in_ch)
        )

        do_np = np.zeros((P, max(c_ovf, 1)), np.float32)
        nv_np = np.zeros((P, max(c_ovf, 1)), np.float32)
        okol = ovf_base[r["ov_w"]] + r["ov_chunk"]
        do_np[r["ov_slot"], okol] = r["ov_off"].astype(np.float32)
        nv_np[r["ov_slot"], okol] = r["ov_dinv"]

        d2_np = np.zeros((P, nw), np.float32)
        nn = cfg.np_per
        loc = np.arange(nn, dtype=np.int64)
        d2_np[loc & 127, loc >> 7] = dinv[base + loc]

        in_maps.append(
            dict(
                stream_t=stream_t,
                do_ovf=do_np,
                nv_ovf=nv_np,
                d2=d2_np,
                iota=iota,
                iotac=iotac,
                w=w_np,
                b=b_np,
            )
        )
    return in_maps


_PROG_CACHE = {}


def _sample_check(out, x, W, b, dinv, s_dst, s_src, n_samples=512, seed=7):
    """Host-recompute a random sample of output rows; returns True if the
    device output matches (guards against rare first-run DMA/engine races)."""
    n = out.shape[0]
    rng = np.random.default_rng(seed)
    samp = rng.choice(n, size=n_samples, replace=False)
    x32 = np.asarray(x, dtype=np.float32)
    w32 = np.asarray(W, dtype=np.float32)
    b32 = np.asarray(b, dtype=np.float32)
    starts = np.searchsorted(s_dst, samp)
    ends = np.searchsorted(s_dst, samp + 1)
    for d, lo, hi in zip(samp, starts, ends):
        srcs = s_src[lo:hi]
        agg = (x32[srcs] * dinv[srcs][:, None]).sum(axis=0) * dinv[d]
        exp = np.maximum(agg @ w32 + b32, 0.0)
        scale = max(float(np.linalg.norm(exp)), 1e-3)
        if float(np.linalg.norm(out[d] - exp)) > 0.02 * scale:
            return False
    return True


def kernel(x, edge_index, W, b):
    cfg = FULL
    k_ovf, per_core, dinv = route_edges(edge_index, cfg)
    aux = per_core[cfg.m]  # s_dst/s_src appended by route_edges
    key = (tuple(int(v) for v in k_ovf), cfg)
    if key not in _PROG_CACHE:
        _PROG_CACHE[key] = build_program(k_ovf, cfg)
    nc = _PROG_CACHE[key]
    in_maps = make_in_maps(x, W, b, k_ovf, per_core, dinv, cfg)
    out = np.empty((cfg.n_nodes, cfg.out_ch), np.float32)
    for attempt in range(3):
        res = run_bass_kernel_spmd(nc, in_maps, core_ids=list(range(cfg.m)))
        for p in range(cfg.m):
            out[p * cfg.np_per : (p + 1) * cfg.np_per] = (
                res.results[p]["out_t"][:, : cfg.np_per].T.astype(np.float32)
            )
        if _sample_check(out, x, W, b, dinv, aux["s_dst"], aux["s_src"]):
            break
        print(f"kernel: sample check failed (attempt {attempt}), re-running", flush=True)
    return out
